# revision 17
# baseline (speedup 1.0000x reference)
"""Trainium2 Bass kernel for varlen (ragged) BERT self-attention.

Strategy: tensor-parallel over heads. 16 heads across 8 NeuronCores ->
2 heads per core. Every core runs an IDENTICAL program (SPMD) on:
  - xt:   full hidden_states, pre-transposed+cast to bf16 on host, (1024, nnz)
  - wt:   this core's slice of Wqkv (q/k/v rows of its 2 heads), as
          matmul-lhsT blocks (8, 128, 384) bf16
  - bias: this core's bias slice (3, 128) f32
Output per core: (128, nnz) bf16 = the 2 owned heads' output rows,
TRANSPOSED. Host transposes back and concatenates (symmetric with the
host-side input pre-transpose).

On-chip per core:
  1. QKV projection: Y^T[384, nnz] = Wc @ X^T, K=1024 in 8 chunks,
     bias added during PSUM->SBUF eviction (DVE tensor_scalar add),
     cast bf16. Gives qT/kT/vT resident in SBUF as [128(=2hx64), nnz].
  2. Attention per "unit" (a sequence, or a pack of small consecutive
     sequences), per head: scoresT[k,q] = kT.T @ qT (K=64); for packs a
     second rank-4 matmul accumulates -10000 into cross-sequence score
     entries (mask rows mk/mq) so exp underflows to zero. exp on ACT
     (1/sqrt(64) folded into the activation scale), then out^T[65, q]
     accumulated as (v|ones).T @ expT -- the ones column yields the
     softmax denominator for free. Normalize stays in the TRANSPOSED
     layout: DVE reciprocal of the denominator row, GpSimd
     partition_broadcast, DVE tensor_tensor multiply (cast bf16), then
     a clean row-contiguous DMA to out^T. No PE transposes on the
     output path at all.

Scheduling: the QKV chunk stream (PE-only work) is interleaved into
attention emission, AND attention itself runs as TWO concurrent unit
streams (head-sequential within a unit to fit PSUM). While one
stream's pair waits on its exp, the other stream's scores/AV matmuls
keep the PE instruction stream dense -- otherwise the HAM clock gate
re-throttles the tensor engine to half clock (k=4/8) during
attention-heavy stretches.

No padding: every sequence is processed at its true length.
"""

import functools
import sys

import numpy as np

for _p in ("/opt/trn_rl_repo",):
    if _p not in sys.path:
        sys.path.append(_p)

import ml_dtypes  # noqa: E402

N_HEADS = 16
HEAD_DIM = 64
DIM = 1024
N_CORES = 8
HEADS_PER_CORE = N_HEADS // N_CORES  # 2

PACK_MAX_LEN = 768  # pack adjacent seqs <=512 up to this many tokens
PACK_MAX_SEQS = 4  # rank of the additive mask term


def _make_units(lengths):
    """Group sequences into attention units: [(offset, L, [seq len list])]."""
    units = []
    off = 0
    cur = None  # (start, [lens])
    for L in lengths:
        if L == 0:
            continue
        if L <= 512:
            if (
                cur is not None
                and sum(cur[1]) + L <= PACK_MAX_LEN
                and len(cur[1]) < PACK_MAX_SEQS
            ):
                cur[1].append(L)
            else:
                if cur is not None:
                    units.append((cur[0], sum(cur[1]), list(cur[1])))
                cur = (off, [L])
        else:
            if cur is not None:
                units.append((cur[0], sum(cur[1]), list(cur[1])))
                cur = None
            units.append((off, L, [L]))
        off += L
    if cur is not None:
        units.append((cur[0], sum(cur[1]), list(cur[1])))
    return units


@functools.lru_cache(maxsize=4)
def _build(nnz, lengths):
    """Build + compile the SPMD Bass program for the given ragged lengths."""
    import concourse.mybir as mybir
    import concourse.tile as tile
    from concourse import bacc
    from concourse.masks import make_identity

    f32 = mybir.dt.float32
    bf16 = mybir.dt.bfloat16
    Exp = mybir.ActivationFunctionType.Exp
    Mult = mybir.AluOpType.mult

    KC = DIM // 128  # 8 contraction chunks
    M3 = 3 * HEADS_PER_CORE * HEAD_DIM  # 384 output dims per core
    D = HEAD_DIM
    HP = HEADS_PER_CORE

    nc = bacc.Bacc("TRN2", target_bir_lowering=False, debug=False)
    xt = nc.declare_dram_parameter("xt", [DIM, nnz], bf16, isOutput=False)
    wt = nc.declare_dram_parameter("wt", [KC, 128, M3], bf16, isOutput=False)
    bias = nc.declare_dram_parameter("bias", [3, 128], f32, isOutput=False)
    # transposed output: rows = 2 heads x 64 dims, cols = tokens
    out = nc.declare_dram_parameter("out", [128, nnz], bf16, isOutput=True)

    units = _make_units(lengths)
    n_tok_chunks = (nnz + 511) // 512

    with tile.TileContext(nc) as tc:
        with (
            tc.tile_pool(name="res", bufs=1) as res,
            tc.tile_pool(name="xp", bufs=4) as xp,
            tc.tile_pool(name="esp", bufs=8) as esp,
            tc.tile_pool(name="rsp", bufs=4) as rsp,
            tc.tile_pool(name="rbp", bufs=4) as rbp,
            tc.tile_pool(name="obp", bufs=4) as obp,
            tc.tile_pool(name="ps", bufs=1, space="PSUM") as ps,
        ):
            xt_view0 = xt[:, :].rearrange("(a p) n -> p a n", p=128)
            # prefetch the first two token chunks on the DVE hardware DMA
            # queue so they transfer in parallel with wt/bias on the sync
            # queue -- this is the critical path to the first matmul
            prefetched = {}
            for ti in (n_tok_chunks - 1, n_tok_chunks - 2):
                t0 = ti * 512
                nt = min(512, nnz - t0)
                xt_t = xp.tile([128, KC, 512], bf16, tag="xt", name="xt_t")
                nc.scalar.dma_start(
                    xt_t[:, :, :nt], xt_view0[:, :, t0 : t0 + nt]
                )
                prefetched[ti] = xt_t

            # --- constants / resident tensors ---
            wt_sb = res.tile([128, KC, M3], bf16)
            nc.sync.dma_start(wt_sb[:], wt[:, :, :].rearrange("a p m -> p a m"))
            bias_sb = res.tile([128, 3], f32)
            nc.sync.dma_start(bias_sb[:], bias[:, :].rearrange("a p -> p a"))
            ident_bf = res.tile([128, 128], bf16)
            make_identity(nc, ident_bf[:])

            qT = res.tile([128, nnz], bf16)
            kT = res.tile([128, nnz], bf16)
            vT = res.tile([128, nnz], bf16)
            qkvT = (qT, kT, vT)

            # persistent v_aug slots, one set per concurrent unit stream:
            # [ktok(128), ones(1)+pad(63)+v(64)]. The ones column is col 0 so
            # the softmax denominator lands on PSUM partition 0 (where
            # reciprocal_approx_fast works); v dims sit at cols 64..127 so
            # the normalize multiply reads PSUM partitions 64..127 (engine
            # partition access must start at 0/64). Ones/pad written once;
            # the v part is refreshed per (unit, head).
            max_nk = max((u[1] + 127) // 128 for u in units)
            va_slots = {}
            for sid in range(2):
                for jc in range(max_nk):
                    va = res.tile([128, 128], bf16, name=f"va{sid}_{jc}")
                    nc.gpsimd.memset(va[:, :], 0.0)
                    nc.gpsimd.memset(va[:, 0:1], 1.0)
                    va_slots[(sid, jc)] = va

            # --- pack mask rows: score += sum_r mk[r,i] * mq[r,j] ---
            # mk[r,i] = 100 on pack-local seq r's keys, else 0
            # mq[r,j] = 0 on pack-local seq r's queries, else -100
            # => cross-sequence entries within a pack get -10000.
            pack_hi = max(
                (u[0] + u[1] for u in units if len(u[2]) > 1), default=0
            )
            if pack_hi:
                # 32 partitions for gpsimd alignment; matmuls read rows 0:4
                mk = res.tile([32, pack_hi], bf16)
                mq = res.tile([32, pack_hi], bf16)
                nc.gpsimd.memset(mk[:, :], 0.0)
                nc.gpsimd.memset(mq[:, :], 0.0)
                for O, Lp, ls in units:
                    if len(ls) < 2:
                        continue
                    nc.gpsimd.memset(mq[:, O : O + Lp], -100.0)
                    so = O
                    for r, L in enumerate(ls):
                        # row r gets 100 (mk) / 0 (mq) on seq r's columns:
                        # predicate (partition - r) != 0 keeps old value
                        nc.gpsimd.affine_select(
                            out=mk[:, so : so + L],
                            in_=mk[:, so : so + L],
                            compare_op=mybir.AluOpType.not_equal,
                            fill=100.0,
                            base=-r,
                            pattern=[[0, L]],
                            channel_multiplier=1,
                        )
                        nc.gpsimd.affine_select(
                            out=mq[:, so : so + L],
                            in_=mq[:, so : so + L],
                            compare_op=mybir.AluOpType.not_equal,
                            fill=0.0,
                            base=-r,
                            pattern=[[0, L]],
                            channel_multiplier=1,
                        )
                        so += L

            xt_view = xt[:, :].rearrange("(a p) n -> p a n", p=128)

            # --- QKV feeder: yields one (ti, mc) matmul group at a time so
            # attention emission can interleave dense PE work (keeps the HAM
            # clock gate released during ACT-bound attention stretches) ---
            state = {"ti_next": n_tok_chunks}  # smallest fully-emitted chunk

            def _qkv_groups():
                for ti in range(n_tok_chunks - 1, -1, -1):
                    t0 = ti * 512
                    nt = min(512, nnz - t0)
                    if ti in prefetched:
                        xt_tile = prefetched[ti]
                    else:
                        xt_tile = xp.tile(
                            [128, KC, 512], bf16, tag="xt", name="xt_t"
                        )
                        nc.sync.dma_start(
                            xt_tile[:, :, :nt], xt_view[:, :, t0 : t0 + nt]
                        )
                    for mc in range(3):
                        mm = ps.tile([128, 512], f32, tag="mm", bufs=1, name="mm")
                        for kc in range(KC):
                            nc.tensor.matmul(
                                mm[:, :nt],
                                wt_sb[:, kc, mc * 128 : (mc + 1) * 128],
                                xt_tile[:, kc, :nt],
                                start=(kc == 0),
                                stop=(kc == KC - 1),
                            )
                        # evict + bias + cast on DVE
                        nc.vector.tensor_scalar_add(
                            qkvT[mc][:, t0 : t0 + nt],
                            mm[:, :nt],
                            bias_sb[:, mc : mc + 1],
                        )
                        if mc == 2:
                            state["ti_next"] = ti
                        yield

            feeder = _qkv_groups()

            # pacing: spread remaining feeder groups over remaining
            # attention pair-iterations (recomputed each step)
            def _unit_iters(u):
                O, L, ls = u
                nk = (L + 127) // 128
                n = 0
                bounds = []
                so = 0
                for sl in ls:
                    bounds.append((so, so + sl))
                    so += sl

                def seqs_in(a, b):
                    return {
                        i
                        for i, (s0, s1) in enumerate(bounds)
                        if a < s1 and b > s0
                    }

                for qc in range((L + 511) // 512):
                    q0 = qc * 512
                    nq = min(512, L - q0)
                    qs = seqs_in(q0, q0 + nq)
                    act = [
                        jc
                        for jc in range(nk)
                        if seqs_in(jc * 128, min(jc * 128 + 128, L)) & qs
                    ]
                    n += (len(act) + 1) // 2
                return n

            n_groups = 3 * n_tok_chunks
            n_iters = 2 * sum(_unit_iters(u) for u in units)
            pace = {"acc": 0.0, "groups": n_groups, "iters": n_iters}

            def feed(n):
                for _ in range(n):
                    if next(feeder, "done") == "done":
                        break
                    pace["groups"] -= 1

            def feed_cb():
                if pace["iters"] > 0:
                    pace["acc"] += pace["groups"] / pace["iters"]
                pace["iters"] -= 1
                k = min(int(pace["acc"]), pace["groups"])
                if k > 0:
                    pace["acc"] -= k
                    feed(k)
                elif pace["groups"] == 0:
                    # feeder dry: emit PE keepalive matmuls so the HAM clock
                    # gate stays released through the ACT-bound tail
                    for _ in range(2):
                        dm = ps.tile([128, 512], f32, tag="mm", bufs=1, name="dm")
                        nc.tensor.matmul(
                            dm[:, :],
                            wt_sb[:, 0, 0:128],
                            qT[:, 0:512],
                            start=True,
                            stop=True,
                        )

            def unit_stream(O, L, ls, sid):
                """Generator emitting one unit's attention FOR ONE HEAD
                (h = sid), yielding at pair boundaries. Two streams, one per
                head, interleave on the PE queue: their K=64 score matmuls
                target opposite PE row-groups (partitions 0-63 vs 64-127),
                so adjacent pairs execute CONCURRENTLY in the array, and
                each stream's independent work fills the other's exp
                latency."""
                masked = len(ls) > 1
                nk = (L + 127) // 128
                nq5 = (L + 511) // 512
                bounds = []
                so = 0
                for sl in ls:
                    bounds.append((so, so + sl))
                    so += sl

                def seqs_in(a, b):
                    return {
                        i
                        for i, (s0, s1) in enumerate(bounds)
                        if a < s1 and b > s0
                    }

                h = sid
                p0 = D * h
                if True:
                    # refresh this stream's v_aug slots for head h
                    for jc in range(nk):
                        nj = min(128, L - jc * 128)
                        vps = ps.tile([128, D], bf16, tag="tp", bufs=1, name="vps")
                        nc.tensor.transpose(
                            vps[:nj, :D],
                            vT[p0 : p0 + D, O + jc * 128 : O + jc * 128 + nj],
                            ident_bf[p0 : p0 + D, p0 : p0 + D],
                        )
                        nc.vector.tensor_copy(
                            va_slots[(sid, jc)][:nj, 64:128], vps[:nj, :D]
                        )
                    yield
                    for qc in range(nq5):
                        q0 = qc * 512
                        nq = min(512, L - q0)
                        ov = ps.tile(
                            [128, 512], f32, tag=f"ov{sid}", bufs=1, name="ov"
                        )
                        qseqs = seqs_in(q0, q0 + nq)
                        active = [
                            jc
                            for jc in range(nk)
                            if seqs_in(jc * 128, min(jc * 128 + 128, L)) & qseqs
                        ]
                        pairs = [
                            active[i : i + 2] for i in range(0, len(active), 2)
                        ]
                        for pair in pairs:
                            feed_cb()
                            sps = ps.tile(
                                [128, 2, 512], f32, tag="sc", bufs=2, name="sps"
                            )
                            es = esp.tile([128, 2, 512], bf16, tag="es", name="es")
                            njs = []
                            for sl, jc in enumerate(pair):
                                nj = min(128, L - jc * 128)
                                njs.append(nj)
                                kseqs = seqs_in(jc * 128, jc * 128 + nj)
                                need_mask = masked and not (
                                    len(kseqs) == 1 and kseqs == qseqs
                                )
                                nc.tensor.matmul(
                                    sps[:nj, sl, :nq],
                                    kT[
                                        p0 : p0 + D,
                                        O + jc * 128 : O + jc * 128 + nj,
                                    ],
                                    qT[p0 : p0 + D, O + q0 : O + q0 + nq],
                                    start=True,
                                    stop=not need_mask,
                                )
                                if need_mask:
                                    nc.tensor.matmul(
                                        sps[:nj, sl, :nq],
                                        mk[:, O + jc * 128 : O + jc * 128 + nj],
                                        mq[:, O + q0 : O + q0 + nq],
                                        start=False,
                                        stop=True,
                                    )
                            nja = max(njs)
                            if len(pair) == 2:
                                nc.scalar.activation(
                                    es[:nja, :, :nq],
                                    sps[:nja, :, :nq],
                                    Exp,
                                    scale=0.125,
                                )
                            else:
                                nc.scalar.activation(
                                    es[:nja, 0, :nq],
                                    sps[:nja, 0, :nq],
                                    Exp,
                                    scale=0.125,
                                )
                            for sl, jc in enumerate(pair):
                                nj = njs[sl]
                                nc.tensor.matmul(
                                    ov[:, :nq],
                                    va_slots[(sid, jc)][:nj, :],
                                    es[:nj, sl, :nq],
                                    start=(jc == active[0]),
                                    stop=(jc == active[-1]),
                                )
                            yield
                        # normalize in transposed layout + row-contiguous DMA:
                        # recip of the den row (PSUM partition 0), broadcast
                        # on Pool, multiply the v rows (PSUM partitions
                        # 64..127) on DVE. The mult must NOT be on gpsimd:
                        # mixing gpsimd op families mid-kernel forces ~5.5us
                        # Q7 library swaps.
                        rs = rsp.tile([1, 512], f32, tag="rs", name="rs")
                        nc.vector.reciprocal_approx_fast(
                            rs[:, :nq], ov[0:1, :nq]
                        )
                        rb = rbp.tile([64, 512], f32, tag="rb", name="rb")
                        nc.gpsimd.partition_broadcast(rb[:, :nq], rs[:, :nq])
                        ot = obp.tile([64, 512], bf16, tag="ob", name="ot")
                        nc.vector.tensor_tensor(
                            ot[:, :nq], ov[64:128, :nq], rb[:, :nq], Mult
                        )
                        nc.sync.dma_start(
                            out[p0 : p0 + D, O + q0 : O + q0 + nq], ot[:, :nq]
                        )
                        yield

            # --- interleaved two-stream (one per head) emission ---
            # chunks back-to-front via the feeder; a unit is ready once all
            # chunks covering [O, O+L) are emitted, i.e. O >= 512*ti_next.
            # Both streams walk the same unit order.
            pending = sorted(units, key=lambda u: u[0], reverse=True)
            pack_idx = [i for i, u in enumerate(pending) if len(u[2]) > 1]
            if pack_idx and pack_idx[0] > 0:
                # move the unit just before the first pack to the very end:
                # its chunks are long emitted, so it keeps the tail (which
                # has no feeder filler left) supplied with independent work
                tail_u = pending.pop(pack_idx[0] - 1)
                pending.append(tail_u)

            iters = [iter(pending), iter(pending)]

            def start_next(sid):
                u = next(iters[sid], None)
                if u is None:
                    return None
                while state["ti_next"] * 512 > u[0]:
                    feed(1)
                return unit_stream(*u, sid)

            streams = [start_next(0), start_next(1)]
            while any(s is not None for s in streams):
                for sid in range(2):
                    s = streams[sid]
                    if s is None:
                        continue
                    if next(s, "done") == "done":
                        streams[sid] = start_next(sid)
            feed(n_groups)  # drain any leftovers

    nc.compile()
    return nc


def _prepare(hidden_states, Wqkv_weight, Wqkv_bias, cu_seqlens):
    """Host-side sharding prep. Returns (nc, in_maps)."""
    hs = np.asarray(hidden_states, dtype=np.float32)
    W = np.asarray(Wqkv_weight, dtype=np.float32)
    b = np.asarray(Wqkv_bias, dtype=np.float32).reshape(-1)
    cs = np.asarray(cu_seqlens).astype(np.int64).reshape(-1)
    nnz, dim = hs.shape
    assert dim == DIM and W.shape == (3 * DIM, DIM)
    lengths = tuple(int(cs[i + 1] - cs[i]) for i in range(len(cs) - 1))
    assert sum(lengths) == nnz, (lengths, nnz)

    nc = _build(nnz, lengths)

    xt_np = np.ascontiguousarray(hs.T).astype(ml_dtypes.bfloat16)
    in_maps = []
    for c in range(N_CORES):
        r0 = c * HEADS_PER_CORE * HEAD_DIM  # 128c
        rows = []
        biases = []
        for part in range(3):  # q, k, v
            rows.append(W[part * DIM + r0 : part * DIM + r0 + 128, :])
            biases.append(b[part * DIM + r0 : part * DIM + r0 + 128])
        Wc = np.concatenate(rows, axis=0)  # (384, 1024)
        wt_np = np.ascontiguousarray(Wc.T.reshape(DIM // 128, 128, 384)).astype(
            ml_dtypes.bfloat16
        )
        bias_np = np.ascontiguousarray(np.stack(biases, axis=0))  # (3, 128)
        in_maps.append({"xt": xt_np, "wt": wt_np, "bias": bias_np})
    return nc, in_maps


def kernel(hidden_states, Wqkv_weight, Wqkv_bias, cu_seqlens, max_seqlen=None):
    from concourse.bass_utils import run_bass_kernel_spmd

    nc, in_maps = _prepare(hidden_states, Wqkv_weight, Wqkv_bias, cu_seqlens)
    res = run_bass_kernel_spmd(nc, in_maps, list(range(N_CORES)))
    # per-core transposed (128, nnz) bf16 -> full (nnz, 1024) f32
    nnz = hidden_states.shape[0]
    out = np.empty((nnz, DIM), dtype=np.float32)
    for c in range(N_CORES):
        out[:, c * 128 : (c + 1) * 128] = (
            np.asarray(res.results[c]["out"]).astype(np.float32).T
        )
    return out
